# revision 12
# baseline (speedup 1.0000x reference)
"""DARNN (dual-attention RNN) Trainium2 Bass kernel.

Strategy (pure data parallel, 8 cores, B=1024 -> 128 samples/core):

All activations are kept feature-major on-chip: a tensor x[b, f] lives in
SBUF as xT[f, b] with features on partitions and the 128 local batch
elements on the free dim. Every matmul then contracts over the partition
dim with the (pre-transposed, bf16) weight as the stationary operand and
the activation as the moving operand (N = 128).

Key algebraic restructuring: the decoder attention output feat_i is only
ever consumed through linear projections (dec_Wc row and dec_Wf row), so
it is never materialized. Instead q_c[t',b] = hs[b,t',:] @ Wc[0,1:] and
q_f[t',b] = hs[b,t',:] @ Wf[0,HD:] are accumulated during the encoder
(one tiny matmul + DMA per step), and each decoder step only needs
  dot_c[b] = sum_t' e[t',b] q_c[t',b] / Z[b]
computed with an elementwise multiply and a ones-vector matmul reduction
over partitions. The softmax max-subtraction is skipped (scores are
O(1), exp is safe).

Encoder LSTM hidden states hs are stored feature-major in SBUF (bf16) and
reused as decoder attention values. feat is pre-transposed and bf16-cast
on the host and streamed per-step; all weights are packed on the host
into their exact SBUF layouts.
"""

import numpy as np
import ml_dtypes

B, T, NF, HE, HD = 1024, 256, 128, 256, 256
NCORES = 8
BL = B // NCORES  # 128 local batch
TP = T + 1  # 257
BF16 = ml_dtypes.bfloat16

_CACHE = {}


def _bf(x):
    return np.ascontiguousarray(x.astype(BF16))


def _f32(x):
    return np.ascontiguousarray(x.astype(np.float32))


def _pack_inputs(inputs):
    """Pack weights/biases into per-SBUF-tile layouts (shared across cores)."""
    f = {k: np.asarray(v, dtype=np.float32) for k, v in inputs.items()}
    p = {}

    # --- encoder attn1: a = tanh(cat[x,h,c] @ Wa1.T + ba1) ---
    # lhsT tile [128, 5, 257]: [p, k, m] = Wa1[m, k*128+p]
    wa1 = f["enc_Wa1"]  # [257, 640]
    p["w_enc_a1"] = _bf(wa1.T.reshape(5, 128, TP).transpose(1, 0, 2))
    b1 = np.zeros((128, 3), np.float32)
    b1.T.flat[:TP] = f["enc_ba1"]  # [p, j] = ba1[j*128+p]
    p["b_enc_a1"] = _f32(b1)

    # --- encoder attn2: alpha = a @ Wa2.T + ba2 ---  K=257 (3 chunks), M=128
    wa2 = f["enc_Wa2"]  # [128, 257]
    w = np.zeros((128, 3, 128), np.float32)
    w2 = wa2.T  # [257, 128]
    w[:, 0, :] = w2[0:128]
    w[:, 1, :] = w2[128:256]
    w[0, 2, :] = w2[256]
    p["w_enc_a2"] = _bf(w)
    p["b_enc_a2"] = _f32(f["enc_ba2"].reshape(128, 1))

    # --- encoder gates: g = Wih@xi + Whh@h + b ---  K chunks: [xi, h0, h1]
    w = np.zeros((128, 3, 4 * HE), np.float32)
    w[:, 0, :] = f["enc_Wih"].T  # [128, 1024]
    w[:, 1, :] = f["enc_Whh"].T[0:128]
    w[:, 2, :] = f["enc_Whh"].T[128:256]
    p["w_enc_g"] = _bf(w)
    bsum = f["enc_bih"] + f["enc_bhh"]
    p["b_enc_g"] = _f32(bsum.reshape(8, 128).T)  # [p, j] = b[j*128+p]

    # --- q projections: q_c = h . Wc[0,1:], q_f = h . Wf[0,HD:] ---
    w = np.zeros((128, 2, 2), np.float32)
    w[:, 0, 0] = f["dec_Wc"][0, 1 : 1 + 128]
    w[:, 1, 0] = f["dec_Wc"][0, 129 : 1 + 256]
    w[:, 0, 1] = f["dec_Wf"][0, HD : HD + 128]
    w[:, 1, 1] = f["dec_Wf"][0, HD + 128 : HD + 256]
    p["w_q"] = _bf(w)

    # --- decoder attn1: a = tanh(cat[h,c,feat] @ Wa1.T + ba1) --- K=768 (6)
    wa1d = f["dec_Wa1"]  # [256, 768]
    p["w_dec_a1"] = _bf(wa1d.T.reshape(6, 128, HE).transpose(1, 0, 2))
    p["b_dec_a1"] = _f32(f["dec_ba1"].reshape(2, 128).T)

    # --- decoder attn2: s = a @ Wa2.T + ba2 --- K=256 (2), M=257
    wa2d = f["dec_Wa2"]  # [257, 256]
    p["w_dec_a2"] = _bf(wa2d.T.reshape(2, 128, TP).transpose(1, 0, 2))
    b2 = np.zeros((128, 3), np.float32)
    b2.T.flat[:TP] = f["dec_ba2"]
    p["b_dec_a2"] = _f32(b2)

    # --- decoder gates --- K chunks: [xi(K=1), h0, h1]
    w = np.zeros((128, 3, 4 * HD), np.float32)
    w[0, 0, :] = f["dec_Wih"][:, 0]
    w[:, 1, :] = f["dec_Whh"].T[0:128]
    w[:, 2, :] = f["dec_Whh"].T[128:256]
    p["w_dec_g"] = _bf(w)
    bsumd = f["dec_bih"] + f["dec_bhh"]
    p["b_dec_g"] = _f32(bsumd.reshape(8, 128).T)

    # --- final: out = hd . Wf[0,:HD] + dot_f/Z + bf ---
    w = np.zeros((128, 2, 1), np.float32)
    w[:, 0, 0] = f["dec_Wf"][0, 0:128]
    w[:, 1, 0] = f["dec_Wf"][0, 128:256]
    p["w_fh"] = _bf(w)

    # --- scalars: [bc, bf, Wc00, 0] ---
    p["consts"] = _f32(
        np.array([[f["dec_bc"][0], f["dec_bf"][0], f["dec_Wc"][0, 0], 0.0]])
    )

    # --- per-core tensors ---
    feat = f["feat"]  # [B, 257, 128]
    target = f["target"]  # [B, 256]
    per_core = []
    for c in range(NCORES):
        sl = slice(c * BL, (c + 1) * BL)
        # featT [f=128, t=257, b=128]
        featT = _bf(feat[sl].transpose(2, 1, 0))
        per_core.append({"featT": featT, "targetT": _f32(target[sl].T)})
    return p, per_core


def _build(enc_steps=TP, dec_steps=T, zero_unused=False, dbg=False):
    import concourse.mybir as mybir
    from concourse import bacc
    from concourse.tile import TileContext

    dt = mybir.dt
    AF = mybir.ActivationFunctionType
    OP = mybir.AluOpType

    nc = bacc.Bacc("TRN2")

    # ---- DRAM parameters ----
    dram = {}

    def din(name, shape, dtype):
        dram[name] = nc.declare_dram_parameter(name, list(shape), dtype, isOutput=False)

    din("featT", (128, TP, BL), dt.bfloat16)
    din("targetT", (T, BL), dt.float32)
    din("w_enc_a1", (128, 5, TP), dt.bfloat16)
    din("b_enc_a1", (128, 3), dt.float32)
    din("w_enc_a2", (128, 3, 128), dt.bfloat16)
    din("b_enc_a2", (128, 1), dt.float32)
    din("w_enc_g", (128, 3, 4 * HE), dt.bfloat16)
    din("b_enc_g", (128, 8), dt.float32)
    din("w_q", (128, 2, 2), dt.bfloat16)
    din("w_dec_a1", (128, 6, HE), dt.bfloat16)
    din("b_dec_a1", (128, 2), dt.float32)
    din("w_dec_a2", (128, 2, TP), dt.bfloat16)
    din("b_dec_a2", (128, 3), dt.float32)
    din("w_dec_g", (128, 3, 4 * HD), dt.bfloat16)
    din("b_dec_g", (128, 8), dt.float32)
    din("w_fh", (128, 2, 1), dt.bfloat16)
    din("consts", (1, 4), dt.float32)
    out_d = nc.declare_dram_parameter("out", [BL], dt.float32, isOutput=True)
    dbg_d = {}
    if dbg:
        for name, shape, dty in [
            ("dbg_hs0", [128, 8, BL], dt.bfloat16),
            ("dbg_q", [128, 3, 2, BL], dt.bfloat16),
            ("dbg_req", [128, 3, 2, BL], dt.bfloat16),
            ("dbg_zd", [1, 2, BL], dt.float32),
            ("dbg_rz", [1, BL], dt.float32),
            ("dbg_dcz", [1, BL], dt.float32),
            ("dbg_xi1", [1, BL], dt.float32),
            ("dbg_y", [1, BL], dt.float32),
            ("dbg_hd", [128, 2, BL], dt.bfloat16),
            ("dbg_aT", [128, 2, BL], dt.bfloat16),
            ("dbg_rf", [128, 3, BL], dt.bfloat16),
            ("dbg_fin", [1, 2, BL], dt.float32),
            ("dbg_dfz", [1, BL], dt.float32),
        ]:
            dbg_d[name] = nc.declare_dram_parameter(name, shape, dty, isOutput=True)

    with TileContext(nc) as tc:
        with (
            tc.tile_pool(name="consts", bufs=1) as cp,
            tc.tile_pool(name="state", bufs=1) as sp,
            tc.tile_pool(name="feat", bufs=8) as fp,
            tc.tile_pool(name="work", bufs=2) as wp,
            tc.tile_pool(name="lstm", bufs=2) as lp,
        ):
            # ---- load weights into SBUF ----
            sb = {}
            for name, shape, dty in [
                ("w_enc_a1", (128, 5, TP), dt.bfloat16),
                ("b_enc_a1", (128, 3), dt.float32),
                ("w_enc_a2", (128, 3, 128), dt.bfloat16),
                ("b_enc_a2", (128, 1), dt.float32),
                ("w_enc_g", (128, 3, 4 * HE), dt.bfloat16),
                ("b_enc_g", (128, 8), dt.float32),
                ("w_q", (128, 2, 2), dt.bfloat16),
                ("w_dec_a1", (128, 6, HE), dt.bfloat16),
                ("b_dec_a1", (128, 2), dt.float32),
                ("w_dec_a2", (128, 2, TP), dt.bfloat16),
                ("b_dec_a2", (128, 3), dt.float32),
                ("w_dec_g", (128, 3, 4 * HD), dt.bfloat16),
                ("b_dec_g", (128, 8), dt.float32),
                ("w_fh", (128, 2, 1), dt.bfloat16),
                ("consts", (1, 4), dt.float32),
            ]:
                t = cp.tile(list(shape), dty, tag=name)
                nc.sync.dma_start(out=t, in_=dram[name].ap())
                sb[name] = t

            ones_bf = cp.tile([128, 1], dt.bfloat16, tag="ones")
            nc.vector.memset(ones_bf, 1.0)
            zero_bf = cp.tile([128, BL], dt.bfloat16, tag="zero")
            nc.vector.memset(zero_bf, 0.0)

            # persistent big buffers
            hs0 = cp.tile([128, TP, BL], dt.bfloat16, tag="hs0")  # h feats 0:128
            hs1 = cp.tile([128, TP, BL], dt.bfloat16, tag="hs1")  # h feats 128:256
            qT = cp.tile([128, 3, 2, BL], dt.bfloat16, tag="qT")  # [t'%128, t'//128, {c,f}, b]
            if zero_unused:
                nc.vector.memset(hs0, 0.0)
                nc.vector.memset(hs1, 0.0)
                nc.vector.memset(qT, 0.0)

            # encoder state
            c_f = sp.tile([128, 2, BL], dt.float32, tag="c_f")
            c_b = sp.tile([128, 2, BL], dt.bfloat16, tag="c_b")
            nc.vector.memset(c_f, 0.0)
            nc.vector.memset(c_b, 0.0)

            with (
                tc.tile_pool(name="ps_a1", bufs=2, space="PSUM") as ps_a1,
                tc.tile_pool(name="ps_g", bufs=2, space="PSUM") as ps_g,
                tc.tile_pool(name="ps_q", bufs=2, space="PSUM") as ps_q,
            ):
                for t in range(enc_steps):
                    ft = fp.tile([128, BL], dt.bfloat16, tag="ft")
                    nc.sync.dma_start(out=ft, in_=dram["featT"].ap()[:, t, :])

                    if t == 0:
                        hp0, hp1 = zero_bf, zero_bf
                    else:
                        hp0, hp1 = hs0[:, t - 1, :], hs1[:, t - 1, :]
                    rhs_a1 = [ft, hp0, hp1, c_b[:, 0, :], c_b[:, 1, :]]

                    # attn1: aT [257 -> (128,128,1), b]
                    a_ps = ps_a1.tile([128, 4, BL], dt.float32, tag="a_ps")
                    for m, mm in enumerate((128, 128, 1)):
                        for k in range(5):
                            nc.tensor.matmul(
                                a_ps[:mm, m, :],
                                sb["w_enc_a1"][:, k, m * 128 : m * 128 + mm],
                                rhs_a1[k],
                                start=(k == 0),
                                stop=(k == 4),
                            )
                    aT = wp.tile([128, 3, BL], dt.bfloat16, tag="aT")
                    for m, mm in enumerate((128, 128, 1)):
                        nc.scalar.activation(
                            out=aT[:mm, m, :],
                            in_=a_ps[:mm, m, :],
                            func=AF.Tanh,
                            bias=sb["b_enc_a1"][:mm, m : m + 1],
                        )

                    # attn2 + xi = (alpha + ba2) * x_t
                    al_ps = a_ps[:, 3, :]
                    for k, kk in enumerate((128, 128, 1)):
                        nc.tensor.matmul(
                            al_ps,
                            sb["w_enc_a2"][:kk, k, :],
                            aT[:kk, k, :],
                            start=(k == 0),
                            stop=(k == 2),
                        )
                    xiT = wp.tile([128, BL], dt.bfloat16, tag="xiT")
                    nc.vector.scalar_tensor_tensor(
                        out=xiT,
                        in0=al_ps,
                        scalar=sb["b_enc_a2"][:, 0:1],
                        in1=ft,
                        op0=OP.add,
                        op1=OP.mult,
                    )

                    # gates
                    g_ps = ps_g.tile([128, 8, BL], dt.float32, tag="g_ps")
                    rhs_g = [xiT, hp0, hp1]
                    for j in range(8):
                        for k in range(3):
                            nc.tensor.matmul(
                                g_ps[:, j, :],
                                sb["w_enc_g"][:, k, j * 128 : (j + 1) * 128],
                                rhs_g[k],
                                start=(k == 0),
                                stop=(k == 2),
                            )

                    # LSTM elementwise (tanh grouped before sigmoids)
                    hdst = (hs0, hs1)
                    bg = sb["b_enc_g"]
                    si = lp.tile([128, 2, BL], dt.float32, tag="si")
                    sf = lp.tile([128, 2, BL], dt.float32, tag="sf")
                    tg = lp.tile([128, 2, BL], dt.float32, tag="tg")
                    so = lp.tile([128, 2, BL], dt.float32, tag="so")
                    for j2 in range(2):
                        nc.scalar.activation(out=tg[:, j2, :], in_=g_ps[:, 4 + j2, :], func=AF.Tanh, bias=bg[:, 4 + j2 : 5 + j2])
                    for j2 in range(2):
                        nc.scalar.activation(out=si[:, j2, :], in_=g_ps[:, 0 + j2, :], func=AF.Sigmoid, bias=bg[:, 0 + j2 : 1 + j2])
                        nc.scalar.activation(out=sf[:, j2, :], in_=g_ps[:, 2 + j2, :], func=AF.Sigmoid, bias=bg[:, 2 + j2 : 3 + j2])
                        nc.scalar.activation(out=so[:, j2, :], in_=g_ps[:, 6 + j2, :], func=AF.Sigmoid, bias=bg[:, 6 + j2 : 7 + j2])
                    p1 = lp.tile([128, 2, BL], dt.float32, tag="p1")
                    tc_ = lp.tile([128, 2, BL], dt.float32, tag="tc")
                    nc.vector.tensor_mul(p1, si, tg)
                    nc.vector.tensor_mul(si, sf, c_f)
                    nc.vector.tensor_add(c_f, si, p1)
                    nc.vector.tensor_copy(c_b, c_f)
                    nc.scalar.activation(out=tc_, in_=c_f, func=AF.Tanh)
                    for j2 in range(2):
                        nc.vector.tensor_mul(hdst[j2][:, t, :], so[:, j2, :], tc_[:, j2, :])

                    # q rows: q_{c,f}[t] = h_t . W -> [2, b] -> DMA to qT row
                    q_ps = ps_q.tile([2, BL], dt.float32, tag="q_ps")
                    for k in range(2):
                        nc.tensor.matmul(
                            q_ps,
                            sb["w_q"][:, k, :],
                            hdst[k][:, t, :],
                            start=(k == 0),
                            stop=(k == 1),
                        )
                    q_row = fp.tile([2, BL], dt.bfloat16, tag="q_row")
                    nc.vector.tensor_copy(q_row, q_ps)
                    nc.sync.dma_start(
                        out=qT[t % 128 : t % 128 + 1, t // 128, :, :], in_=q_row
                    )

            # ================= decoder =================
            hdT = sp.tile([128, 2, BL], dt.bfloat16, tag="hdT")
            cd_f = sp.tile([128, 2, BL], dt.float32, tag="cd_f")
            cd_b = sp.tile([128, 2, BL], dt.bfloat16, tag="cd_b")
            nc.vector.memset(hdT, 0.0)
            nc.vector.memset(cd_f, 0.0)
            nc.vector.memset(cd_b, 0.0)
            o_sb = sp.tile([1, BL], dt.float32, tag="o_sb")

            with (
                tc.tile_pool(name="ps_da1", bufs=2, space="PSUM") as ps_da1,
                tc.tile_pool(name="ps_s", bufs=1, space="PSUM") as ps_s,
                tc.tile_pool(name="ps_zd", bufs=1, space="PSUM") as ps_zd,
                tc.tile_pool(name="ps_dg", bufs=2, space="PSUM") as ps_dg,
            ):
                for t in range(dec_steps):
                    rhs_a1 = [
                        hdT[:, 0, :],
                        hdT[:, 1, :],
                        cd_b[:, 0, :],
                        cd_b[:, 1, :],
                        hs0[:, t, :],
                        hs1[:, t, :],
                    ]
                    a_ps = ps_da1.tile([128, 2, BL], dt.float32, tag="da_ps")
                    for m in range(2):
                        for k in range(6):
                            nc.tensor.matmul(
                                a_ps[:, m, :],
                                sb["w_dec_a1"][:, k, m * 128 : (m + 1) * 128],
                                rhs_a1[k],
                                start=(k == 0),
                                stop=(k == 5),
                            )
                    aT = wp.tile([128, 2, BL], dt.bfloat16, tag="daT")
                    for m in range(2):
                        nc.scalar.activation(
                            out=aT[:, m, :],
                            in_=a_ps[:, m, :],
                            func=AF.Tanh,
                            bias=sb["b_dec_a1"][:, m : m + 1],
                        )

                    # attn2 scores s [257 -> (128,128,1), b]
                    s_ps = ps_s.tile([128, 3, BL], dt.float32, tag="s_ps")
                    for m, mm in enumerate((128, 128, 1)):
                        for k in range(2):
                            nc.tensor.matmul(
                                s_ps[:mm, m, :],
                                sb["w_dec_a2"][:, k, m * 128 : m * 128 + mm],
                                aT[:, k, :],
                                start=(k == 0),
                                stop=(k == 1),
                            )

                    # e = exp(s + ba2); eq = e * q_c
                    req = wp.tile([128, 3, 2, BL], dt.bfloat16, tag="req")
                    for m, mm in enumerate((128, 128, 1)):
                        nc.scalar.activation(
                            out=req[:mm, m, 0, :],
                            in_=s_ps[:mm, m, :],
                            func=AF.Exp,
                            bias=sb["b_dec_a2"][:mm, m : m + 1],
                        )
                    for m, mm in enumerate((128, 128, 1)):
                        nc.vector.tensor_mul(
                            req[:mm, m, 1, :], req[:mm, m, 0, :], qT[:mm, m, 0, :]
                        )

                    # [Z | dot_c] = ones . [e | eq]
                    zd_ps = ps_zd.tile([1, 2, BL], dt.float32, tag="zd_ps")
                    for m, mm in enumerate((128, 128, 1)):
                        nc.tensor.matmul(
                            zd_ps,
                            ones_bf[:mm, :],
                            req[:mm, m, :, :],
                            start=(m == 0),
                            stop=(m == 2),
                        )

                    # xi = y_t*Wc00 + dot_c/Z + bc
                    zd_cp = wp.tile([1, 2, BL], dt.float32, tag="zd_cp")
                    if dbg:
                        nc.vector.tensor_copy(zd_cp, zd_ps)
                    rz = wp.tile([1, BL], dt.float32, tag="rz")
                    dcz = wp.tile([1, BL], dt.float32, tag="dcz")
                    xi1 = wp.tile([1, BL], dt.float32, tag="xi1")
                    xiT = wp.tile([1, BL], dt.bfloat16, tag="dxiT")
                    nc.vector.reciprocal(rz, zd_ps[0:1, 0, :])
                    nc.vector.tensor_mul(dcz, zd_ps[0:1, 1, :], rz)
                    y_row = fp.tile([1, BL], dt.float32, tag="y_row")
                    nc.sync.dma_start(out=y_row, in_=dram["targetT"].ap()[t : t + 1, :])
                    nc.vector.scalar_tensor_tensor(
                        out=xi1,
                        in0=y_row,
                        scalar=sb["consts"][0:1, 2:3],
                        in1=dcz,
                        op0=OP.mult,
                        op1=OP.add,
                    )
                    nc.vector.tensor_scalar(
                        out=xiT,
                        in0=xi1,
                        scalar1=sb["consts"][0:1, 0:1],
                        scalar2=None,
                        op0=OP.add,
                    )

                    # gates
                    g_ps = ps_dg.tile([128, 8, BL], dt.float32, tag="dg_ps")
                    for j in range(8):
                        nc.tensor.matmul(
                            g_ps[:, j, :],
                            sb["w_dec_g"][0:1, 0, j * 128 : (j + 1) * 128],
                            xiT,
                            start=True,
                            stop=False,
                        )
                        for k in (1, 2):
                            nc.tensor.matmul(
                                g_ps[:, j, :],
                                sb["w_dec_g"][:, k, j * 128 : (j + 1) * 128],
                                hdT[:, k - 1, :],
                                start=False,
                                stop=(k == 2),
                            )

                    bg = sb["b_dec_g"]
                    si = lp.tile([128, 2, BL], dt.float32, tag="dsi")
                    sf = lp.tile([128, 2, BL], dt.float32, tag="dsf")
                    tg = lp.tile([128, 2, BL], dt.float32, tag="dtg")
                    so = lp.tile([128, 2, BL], dt.float32, tag="dso")
                    for j2 in range(2):
                        nc.scalar.activation(out=tg[:, j2, :], in_=g_ps[:, 4 + j2, :], func=AF.Tanh, bias=bg[:, 4 + j2 : 5 + j2])
                    for j2 in range(2):
                        nc.scalar.activation(out=si[:, j2, :], in_=g_ps[:, 0 + j2, :], func=AF.Sigmoid, bias=bg[:, 0 + j2 : 1 + j2])
                        nc.scalar.activation(out=sf[:, j2, :], in_=g_ps[:, 2 + j2, :], func=AF.Sigmoid, bias=bg[:, 2 + j2 : 3 + j2])
                        nc.scalar.activation(out=so[:, j2, :], in_=g_ps[:, 6 + j2, :], func=AF.Sigmoid, bias=bg[:, 6 + j2 : 7 + j2])
                    p1 = lp.tile([128, 2, BL], dt.float32, tag="dp1")
                    tc_ = lp.tile([128, 2, BL], dt.float32, tag="dtc")
                    nc.vector.tensor_mul(p1, si, tg)
                    nc.vector.tensor_mul(si, sf, cd_f)
                    nc.vector.tensor_add(cd_f, si, p1)
                    nc.vector.tensor_copy(cd_b, cd_f)
                    nc.scalar.activation(out=tc_, in_=cd_f, func=AF.Tanh)
                    for j2 in range(2):
                        nc.vector.tensor_mul(hdT[:, j2, :], so[:, j2, :], tc_[:, j2, :])

                    if t == dec_steps - 1:
                        # dot_f = ones . (e * q_f);  hw = hd . Wf[:, :HD]
                        rf = wp.tile([128, 3, BL], dt.bfloat16, tag="rf")
                        for m, mm in enumerate((128, 128, 1)):
                            nc.vector.tensor_mul(
                                rf[:mm, m, :], req[:mm, m, 0, :], qT[:mm, m, 1, :]
                            )
                        fin_ps = ps_zd.tile([1, 2, BL], dt.float32, tag="zd_ps")
                        for m, mm in enumerate((128, 128, 1)):
                            nc.tensor.matmul(
                                fin_ps[0:1, 0, :],
                                ones_bf[:mm, :],
                                rf[:mm, m, :],
                                start=(m == 0),
                                stop=(m == 2),
                            )
                        for k in range(2):
                            nc.tensor.matmul(
                                fin_ps[0:1, 1, :],
                                sb["w_fh"][:, k, :],
                                hdT[:, k, :],
                                start=(k == 0),
                                stop=(k == 1),
                            )
                        dfz = wp.tile([1, BL], dt.float32, tag="dfz")
                        nc.vector.tensor_mul(dfz, fin_ps[0:1, 0, :], rz)
                        if dbg:
                            fin_cp = wp.tile([1, 2, BL], dt.float32, tag="fin_cp")
                            nc.vector.tensor_copy(fin_cp, fin_ps)
                            nc.sync.dma_start(out=dbg_d["dbg_rf"].ap(), in_=rf)
                            nc.sync.dma_start(out=dbg_d["dbg_fin"].ap(), in_=fin_cp)
                            nc.sync.dma_start(out=dbg_d["dbg_dfz"].ap(), in_=dfz)
                        hw_sb = wp.tile([1, BL], dt.float32, tag="hw_sb")
                        nc.vector.tensor_copy(hw_sb, fin_ps[0:1, 1, :])
                        nc.vector.scalar_tensor_tensor(
                            out=o_sb,
                            in0=hw_sb,
                            scalar=sb["consts"][0:1, 1:2],
                            in1=dfz,
                            op0=OP.add,
                            op1=OP.add,
                        )
                        nc.sync.dma_start(out=out_d.ap(), in_=o_sb[0:1, :])
                        if dbg:
                            nc.sync.dma_start(out=dbg_d["dbg_hs0"].ap(), in_=hs0[:, 0:8, :])
                            nc.sync.dma_start(out=dbg_d["dbg_q"].ap(), in_=qT)
                            nc.sync.dma_start(out=dbg_d["dbg_req"].ap(), in_=req)
                            nc.sync.dma_start(out=dbg_d["dbg_zd"].ap(), in_=zd_cp)
                            nc.sync.dma_start(out=dbg_d["dbg_rz"].ap(), in_=rz)
                            nc.sync.dma_start(out=dbg_d["dbg_dcz"].ap(), in_=dcz)
                            nc.sync.dma_start(out=dbg_d["dbg_xi1"].ap(), in_=xi1)
                            nc.sync.dma_start(out=dbg_d["dbg_y"].ap(), in_=y_row)
                            nc.sync.dma_start(out=dbg_d["dbg_hd"].ap(), in_=hdT)
                            nc.sync.dma_start(out=dbg_d["dbg_aT"].ap(), in_=aT)

    nc.finalize()
    return nc


def _get_nc():
    if "nc" not in _CACHE:
        _CACHE["nc"] = _build()
    return _CACHE["nc"]


def _run(inputs, **kw):
    from concourse.bass_utils import run_bass_kernel_spmd

    shared, per_core = _pack_inputs(inputs)
    nc = _get_nc()
    in_maps = []
    for c in range(NCORES):
        m = dict(shared)
        m.update(per_core[c])
        in_maps.append(m)
    res = run_bass_kernel_spmd(nc, in_maps, list(range(NCORES)), **kw)
    out = np.concatenate([np.asarray(res.results[c]["out"]) for c in range(NCORES)])
    return out.astype(np.float32).reshape(B, 1), res


def kernel(**inputs):
    return _run(inputs)[0]
